# revision 25
# baseline (speedup 1.0000x reference)
"""Trainium2 Bass kernel for nn_HLF_14474039787790.

Math: the reference applies softmax over a singleton axis, which is exactly
1.0, so h = x1 + x1*1 + x2*1 = 2*x1 + x2 and x3 is dead.
    feats[b, c] = (2*sum_s x1[b,c,s] + sum_s x2[b,c,s]) / 49      (B, 2048)
    logits      = feats @ W.T + bias                              (B, 14)

Sharding: pure data parallel over 8 NeuronCores, 16 batches each.

Per-core layout: the shard x[16, 2048, 49] is viewed flat as (128, 256, 49):
partition p holds 256 consecutive (batch, channel) blocks of 49 spatial
values, so every DMA descriptor is a long contiguous run (50 KB/partition).
DVE reduces the innermost 49. feats in SBUF is (128, 256) which flattens to
exactly (16, 2048) row-major.

FC: fully incremental. For each streamed column-chunk, the freshly reduced
sums slice is transposed on TensorE into a base-0 (w, 128) tile, copied to
SBUF on ScalarE, and immediately contracted (8 accumulating matmuls) against
a host-pre-sliced W block in matching layout. x1's matmuls use 2*W and x2's
use W so a single PSUM accumulator yields (2*S1 + S2) @ W^T directly; the
epilogue is one DVE op (P/49 + bias) and tiny DMAs. Chunk widths taper at
the end of the stream so the last reduce barely lags the last DMA.
"""

import numpy as np

import concourse.bacc as bacc
import concourse.mybir as mybir
import concourse.tile as tile
from concourse.bass_utils import run_bass_kernel_spmd
from concourse.masks import make_identity

N_CORES = 8
B = 128
BS = B // N_CORES          # 16 batches per core
C = 2048
S = 49
NBLK = BS * C // 128       # 256 blocks of 49 per partition
O = 14
KCH = C // 128             # 16 contraction chunks for the FC
# (col0, width) column-chunks of the (128, 256) sums layout; widths taper at
# the end so the final reduce lags the final DMA as little as possible.
CHUNKS = [(0, 64), (64, 64), (128, 64), (192, 32), (224, 16), (240, 16)]
NCK = len(CHUNKS)

_CACHE = {}


def _build(loop_iters=1):
    nc = bacc.Bacc(
        "TRN2",
        target_bir_lowering=False,
        debug=False,
        num_devices=N_CORES,
    )
    f32 = mybir.dt.float32
    x1 = nc.dram_tensor("x1s", [128, NBLK, S], f32, kind="ExternalInput").ap()
    x2 = nc.dram_tensor("x2s", [128, NBLK, S], f32, kind="ExternalInput").ap()
    # wtk[q, k, s, i, o] = (2-s) * W[o, ch(k, i, q)]  (host pre-arranged per
    # chunk, re-based to partition 0)
    wtk = nc.dram_tensor(
        "wtk", [64, NCK, 2, KCH // 2, O], f32, kind="ExternalInput"
    ).ap()
    # bias as a single row, accumulated into the FC via a K=1 ones matmul
    bb = nc.dram_tensor("bb", [1, O], f32, kind="ExternalInput").ap()
    logits = nc.dram_tensor("logits", [BS, O], f32, kind="ExternalOutput").ap()
    feats = nc.dram_tensor("feats", [128, NBLK], f32, kind="ExternalOutput").ap()

    with tile.TileContext(nc) as tc:
        with (
            tc.tile_pool(name="xin", bufs=8) as xin,
            tc.tile_pool(name="stpool", bufs=4) as stpool,
            tc.tile_pool(name="persist", bufs=1) as persist,
            tc.tile_pool(name="psum", bufs=1, space="PSUM") as pp,
            tc.tile_pool(name="psum_st", bufs=4, space="PSUM") as pst,
        ):
            if loop_iters > 1:
                loop_cm = tc.For_i(0, loop_iters, 1)
                loop_cm.__enter__()
            # --- constants: tiny DMAs on the ACT HWDGE ring so the SP ring
            # starts streaming x-data immediately ---
            wtk_t = persist.tile([64, NCK, 2, KCH // 2, O], f32, tag="wtk")
            nc.scalar.dma_start(out=wtk_t, in_=wtk)
            bias_t = persist.tile([1, O], f32, tag="bias")
            nc.scalar.dma_start(out=bias_t, in_=bb)
            ident = persist.tile([128, 128], f32, tag="ident")
            make_identity(nc, ident)
            ones_t = persist.tile([1, BS], f32, tag="ones")
            nc.gpsimd.memset(ones_t, 1.0)

            s1 = persist.tile([128, NBLK], f32, tag="s1")
            s2 = persist.tile([128, NBLK], f32, tag="s2")
            p_acc = pp.tile([BS, O], f32, tag="p")

            # bias row: p_acc starts as ones^T @ bias = broadcast bias
            nc.tensor.matmul(p_acc, ones_t, bias_t, start=True, stop=False)
            n_mm = NCK * 2 * (KCH // 2)
            mm = 1
            for k, (col0, w) in enumerate(CHUNKS):
                stbs = []
                # io + reduce + transpose + copy for both tensors first, so
                # the next chunk's transpose never queues behind a matmul
                # burst on PE
                for s_idx, (x, s) in enumerate(((x1, s1), (x2, s2))):
                    t_in = xin.tile([128, 64, S], f32, tag="x", name=f"x_{k}{s_idx}")
                    nc.sync.dma_start(
                        out=t_in[:, :w, :], in_=x[:, col0 : col0 + w, :]
                    )
                    nc.vector.reduce_sum(
                        out=s[:, col0 : col0 + w],
                        in_=t_in[:, :w, :],
                        axis=mybir.AxisListType.X,
                    )
                    stp = pst.tile([64, 128], f32, tag="stp", name=f"stp_{k}{s_idx}")
                    nc.tensor.transpose(
                        stp[:w, :], s[:, col0 : col0 + w], ident
                    )
                    stb = stpool.tile(
                        [64, 128], f32, tag="stb", name=f"stb_{k}{s_idx}"
                    )
                    nc.scalar.copy(stb[:w, :], stp[:w, :])
                    stbs.append(stb)
                for s_idx, stb in enumerate(stbs):
                    stb_v = stb.rearrange("q (b g) -> q b g", g=8)
                    for i in range(KCH // 2):
                        nc.tensor.matmul(
                            p_acc,
                            stb_v[:w, :, i],
                            wtk_t[:w, k, s_idx, i, :],
                            start=False,
                            stop=(mm == n_mm),
                        )
                        mm += 1

            # --- feats = (2*s1 + s2) / 49 (parallel with the FC tail) ---
            f_raw = persist.tile([128, NBLK], f32, tag="f_raw")
            nc.vector.scalar_tensor_tensor(
                out=f_raw,
                in0=s1,
                scalar=2.0,
                in1=s2,
                op0=mybir.AluOpType.mult,
                op1=mybir.AluOpType.add,
            )
            # --- logits: p_acc already holds (2S1+S2)@(W/49)^T + bias ---
            lg_sb = persist.tile([BS, O], f32, tag="lgsb")
            nc.vector.tensor_copy(out=lg_sb, in_=p_acc)
            nc.sync.dma_start(out=logits, in_=lg_sb)
            f_sc = persist.tile([128, NBLK], f32, tag="f_sc")
            nc.scalar.mul(f_sc, f_raw, 1.0 / 49.0)
            nc.scalar.dma_start(out=feats, in_=f_sc)

            if loop_iters > 1:
                loop_cm.__exit__(None, None, None)

    nc.compile()
    return nc


def _get_nc():
    if "nc" not in _CACHE:
        _CACHE["nc"] = _build()
    return _CACHE["nc"]


def _in_maps(x1, x2, W, b):
    x1 = np.ascontiguousarray(x1, dtype=np.float32).reshape(B, C, S)
    x2 = np.ascontiguousarray(x2, dtype=np.float32).reshape(B, C, S)
    W = np.ascontiguousarray(W, dtype=np.float32)
    b = np.ascontiguousarray(b, dtype=np.float32)
    # wtk[q, k, s, i, o] = (2-s) * W[o, ch] with ch = (2*i + col0//128)*128
    # + (col0 % 128) + q  -- the per-chunk, re-based W^T blocks
    # 1/49 (the spatial mean) is folded into the FC weights; bias rides in
    # as a K=1 matmul row.
    wtk = np.zeros((64, NCK, 2, KCH // 2, O), dtype=np.float32)
    for k, (col0, w) in enumerate(CHUNKS):
        h, q0 = col0 // 128, col0 % 128
        for i in range(KCH // 2):
            c = 2 * i + h
            blk = W[:, c * 128 + q0 : c * 128 + q0 + w].T  # (w, O)
            wtk[:w, k, 0, i, :] = (2.0 / 49.0) * blk
            wtk[:w, k, 1, i, :] = (1.0 / 49.0) * blk
    bb = b.reshape(1, O).copy()
    maps = []
    for i in range(N_CORES):
        maps.append(
            {
                "x1s": x1[i * BS : (i + 1) * BS].reshape(128, NBLK, S),
                "x2s": x2[i * BS : (i + 1) * BS].reshape(128, NBLK, S),
                "wtk": wtk,
                "bb": bb,
            }
        )
    return maps


def _run(x1, x2, W, b, **run_kwargs):
    nc = _get_nc()
    res = run_bass_kernel_spmd(
        nc, _in_maps(x1, x2, W, b), core_ids=list(range(N_CORES)), **run_kwargs
    )
    logits = np.concatenate([r["logits"] for r in res.results], axis=0)
    feats = np.concatenate(
        [r["feats"].reshape(BS, C) for r in res.results], axis=0
    )
    return (logits, feats), res


def kernel(x1, x2, x3, W, b):
    (logits, feats), _ = _run(x1, x2, W, b)
    return (logits, feats)


def kernel_traced(x1, x2, x3, W, b, **trace_kwargs):
    """Like kernel() but with NTFF profiling; returns (outputs, BassKernelResults)."""
    return _run(x1, x2, W, b, trace=True, **trace_kwargs)
